# revision 23
# baseline (speedup 1.0000x reference)
"""Trainium2 Bass kernel for nn_DenseTf: out = x @ sign(clip(w,-1,1)) + b.

Shapes (hardcoded from the problem spec):
    x: [8192, 4096] f32, w: [4096, 4096] f32, b: [4096] f32 -> out [8192, 4096] f32

Strategy: data-parallel over tokens across 8 NeuronCores. Each core computes
    out_c [1024, 4096] = x_c [1024, 4096] @ sign(w) [4096, 4096] + b
as a bf16 tensor-engine matmul with fp32 PSUM accumulation:
  - x_c is cast f32->bf16 by a gpsimd (SWDGE) cast-DMA into a DRAM scratch,
    then XBAR-transpose-loaded into a resident SBUF tile xT [128, 32, 1024]
    (partition = d_in%128, mid = d_in//128 block, free = token).
  - w streams in once as 1MB f32 quad tiles [128, 4, 512]; the Scalar engine
    binarizes+casts (Sign activation, f32 in -> bf16 out).
  - matmuls: lhsT (stationary) = xT[:, k, m*128:(m+1)*128], rhs (moving) =
    sign(w) tile [128, 512]; 8 PSUM banks hold the 8 token-tiles of one
    512-wide filter chunk, accumulated over all 32 k-tiles (m-inner, k-outer).
  - bias is folded into the accumulation group as two K=1 matmuls against a
    bf16 hi/lo split of b (exact for b=0, ~fp32-accurate otherwise).
"""

import os

import numpy as np

N_CORES = 8
N_TOKENS = 8192
D_IN = 4096
FILTERS = 4096
P = 128

# Populated by kernel() after each run (BassKernelResults); test harness reads
# exec_time_ns off this.
LAST_RESULT = None

_CACHE = {}


def _build(m_per_core=N_TOKENS // N_CORES, d_in=D_IN, filters=FILTERS, fc=512, kq=4,
           reps=1, rep_xprep=True, rep_bias=True, mm_only=False, no_sign=False,
           ws_bufs=4, wb_bufs=6, out_bufs=6, binarize="dve", xprep="interleave",
           waves=1):
    """Build + compile the single-core Bass program (SPMD across cores).

    reps>1 replicates the whole body inside one NEFF (timing only: wall-clock
    differencing against reps=1 cancels the axon dispatch overhead).
    rep_xprep/rep_bias control whether those phases replicate too (for
    bisection of where HW time goes)."""
    import concourse.mybir as mybir
    import concourse.tile as tile
    from concourse import bacc

    DT = mybir.dt.float16            # matmul dtype (fp16: 1 cyc/row, 10-bit mantissa)
    m_tiles = m_per_core // P        # token tiles of 128
    k_tiles = d_in // P              # contraction tiles of 128
    n_fc = filters // fc             # filter chunks
    n_kq = k_tiles // kq             # w DMA quads per chunk
    q_d = kq * P                     # d_in columns per x-prep chunk (matches kq)
    n_q = d_in // q_d

    nc = bacc.Bacc("TRN2", debug=False, target_bir_lowering=False)

    x_d = nc.dram_tensor("x", [m_per_core, d_in], mybir.dt.float32, kind="ExternalInput")
    w_d = nc.dram_tensor("w", [d_in, filters], mybir.dt.float32, kind="ExternalInput")
    b_d = nc.dram_tensor("b", [filters], mybir.dt.float32, kind="ExternalInput")
    o_d = nc.dram_tensor("out", [m_per_core, filters], mybir.dt.float32, kind="ExternalOutput")

    w_v = w_d[:].rearrange("(ko p) f -> p ko f", p=P)  # [128, k_tiles, filters]

    with tile.TileContext(nc) as tc:
        with (
            tc.tile_pool(name="dram", bufs=1, space="DRAM") as dram_pool,
            tc.tile_pool(name="xt", bufs=1) as xt_pool,
            tc.tile_pool(name="const", bufs=1) as const_pool,
            tc.tile_pool(name="bstage", bufs=2) as bs_pool,
            tc.tile_pool(name="wstage", bufs=ws_bufs) as ws_pool,
            tc.tile_pool(name="wbin", bufs=wb_bufs) as wb_pool,
            tc.tile_pool(name="outs", bufs=out_bufs) as out_pool,
        ):
            state = {}

            def emit_xprep_chunk(q):
                # x prep for one d_in chunk: gpsimd cast-DMA f32->fp16 into
                # DRAM scratch, then XBAR transpose-load (ACT HWDGE ring,
                # parallel to the SP ring carrying w/out).
                xbf, xT = state["xbf"], state["xT"]
                dsl = slice(q * q_d, (q + 1) * q_d)
                nc.gpsimd.dma_start(xbf[:, dsl], x_d[:, dsl])      # SWDGE cast
                nc.scalar.dma_start(
                    xT[:, q * kq:(q + 1) * kq, :], xbf[:, dsl], transpose=True
                )

            def alloc_x_tiles():
                state["xbf"] = dram_pool.tile([m_per_core, d_in], DT, name="xbf")
                state["xT"] = xt_pool.tile([P, k_tiles, m_per_core], DT, name="xT")

            def emit_xprep():
                alloc_x_tiles()
                xbf, xT = state["xbf"], state["xT"]
                for q in range(n_q):
                    dsl = slice(q * q_d, (q + 1) * q_d)
                    nc.gpsimd.dma_start(xbf[:, dsl], x_d[:, dsl])  # SWDGE cast
                for q in range(n_q):
                    dsl = slice(q * q_d, (q + 1) * q_d)
                    nc.scalar.dma_start(
                        xT[:, q * kq:(q + 1) * kq, :], xbf[:, dsl], transpose=True
                    )

            def emit_bias():
                # bias: hi/lo fp16 split, broadcast to [128, filters] via PE
                # (ones[1,128].T @ b[1,:]); runs inside the startup bubble.
                ones_sb = const_pool.tile([1, P], DT, name="ones_sb")
                nc.any.memset(ones_sb[:], 1.0)
                b_hi = const_pool.tile([1, filters], DT, name="b_hi")
                b_lo = const_pool.tile([1, filters], DT, name="b_lo")
                bias_bc = const_pool.tile([P, filters], mybir.dt.float32,
                                          name="bias_bc")
                for i in range(n_fc):
                    sl = slice(i * fc, (i + 1) * fc)
                    bs = bs_pool.tile([1, fc], mybir.dt.float32, tag="bs", name="bs")
                    nc.sync.dma_start(bs[:], b_d[None, sl])
                    nc.vector.tensor_copy(b_hi[:, sl], bs[:])     # hi = fp16(b)
                    bh32 = bs_pool.tile([1, fc], mybir.dt.float32, tag="bh32",
                                        name="bh32")
                    nc.vector.tensor_copy(bh32[:], b_hi[:, sl])
                    nc.vector.tensor_sub(bs[:], bs[:], bh32[:])   # residual
                    nc.vector.tensor_copy(b_lo[:, sl], bs[:])     # lo = fp16(b-hi)
                with tc.tile_pool(name="psum_b", bufs=n_fc, space="PSUM") as psum_b:
                    for i in range(n_fc):
                        sl = slice(i * fc, (i + 1) * fc)
                        pb = psum_b.tile([P, fc], mybir.dt.float32, tag="pb",
                                         name="pb")
                        nc.tensor.matmul(pb[:], ones_sb[:1, :], b_hi[:1, sl],
                                         start=True, stop=False)
                        nc.tensor.matmul(pb[:], ones_sb[:1, :], b_lo[:1, sl],
                                         start=False, stop=True)
                        nc.vector.tensor_copy(bias_bc[:, sl], pb[:])
                state["bias_bc"] = bias_bc

            def emit_wprep(f, qi, wb_const):
                fsl = slice(f * fc, (f + 1) * fc)
                if mm_only:
                    return wb_const
                ws = ws_pool.tile([P, kq, fc], mybir.dt.float32,
                                  tag="ws", name="ws")
                nc.sync.dma_start(ws[:], w_v[:, qi * kq:(qi + 1) * kq, fsl])
                if no_sign:
                    # diagnostic: DMA w but matmul a const tile
                    nc.vector.tensor_copy(ws[:1, :1, :8], ws[:1, :1, :8])
                    return wb_const
                wb = wb_pool.tile([P, kq, fc], DT, tag="wb", name="wb")
                if binarize == "dve":
                    # (w >= 0) - 0.5 -> {-0.5, +0.5}; the *2 folds into evict
                    nc.vector.tensor_scalar(
                        wb[:], ws[:], 0.0, 0.5,
                        mybir.AluOpType.is_ge, mybir.AluOpType.subtract)
                else:
                    nc.scalar.sign(wb[:], ws[:])                  # binarize+cast
                return wb

            def emit_evict(f, m, psum_m):
                fsl = slice(f * fc, (f + 1) * fc)
                bias_bc = state["bias_bc"]
                ot = out_pool.tile([P, fc], mybir.dt.float32, tag="ot", name="ot")
                if binarize == "dve" and not (mm_only or no_sign):
                    # weights were {+-0.5}: out = 2*psum + bias
                    nc.vector.tensor_scalar(ot[:], psum_m[:], 2.0, None,
                                            mybir.AluOpType.mult)
                    nc.vector.tensor_add(ot[:], ot[:], bias_bc[:, fsl])
                else:
                    nc.vector.tensor_add(ot[:], psum_m[:], bias_bc[:, fsl])
                nc.sync.dma_start(o_d[m * P:(m + 1) * P, fsl], ot[:])

            def emit_mm_group(f, qi, wb, psums, wave_ms):
                xT = state["xT"]
                for kk in range(kq):
                    k = qi * kq + kk
                    for m in wave_ms:
                        nc.tensor.matmul(
                            psums[m][:],
                            xT[:, k, m * P:(m + 1) * P],
                            wb[:, kk, :],
                            start=(k == 0),
                            stop=(k == k_tiles - 1),
                        )

            def emit_main():
                # main loop: stream w once, binarize, matmul
                wb_const = None
                if mm_only or no_sign:
                    wb_const = const_pool.tile([P, kq, fc], DT, name="wb_const")
                    nc.any.memset(wb_const[:], 1.0)
                mw = m_tiles // waves
                with tc.tile_pool(name="psum", bufs=m_tiles, space="PSUM") as pp:
                    for f in range(n_fc):
                        psums = {}
                        wbs = {}
                        for wv in range(waves):
                            wave_ms = range(wv * mw, (wv + 1) * mw)
                            for m in wave_ms:
                                psums[m] = pp.tile([P, fc], mybir.dt.float32,
                                                   tag="ps", name=f"ps_{f}_{m}")
                            for qi in range(n_kq):
                                if xprep == "interleave" and f == 0 and wv == 0:
                                    emit_xprep_chunk(qi)
                                if wv == 0:
                                    wbs[qi] = emit_wprep(f, qi, wb_const)
                                emit_mm_group(f, qi, wbs[qi], psums, wave_ms)
                            for m in wave_ms:
                                emit_evict(f, m, psums[m])

            if not rep_xprep:
                if xprep == "interleave":
                    alloc_x_tiles()
                else:
                    emit_xprep()
            if not rep_bias:
                emit_bias()
            for _rep in range(reps):
                if rep_xprep:
                    if xprep == "interleave":
                        alloc_x_tiles()
                    else:
                        emit_xprep()
                if rep_bias:
                    emit_bias()
                emit_main()

    nc.compile()
    return nc


def _get_nc():
    key = "full"
    if key not in _CACHE:
        _CACHE[key] = _build()
    return _CACHE[key]


def kernel(x, w, b):
    global LAST_RESULT
    from concourse.bass_utils import run_bass_kernel_spmd

    x = np.ascontiguousarray(np.asarray(x, dtype=np.float32))
    w = np.ascontiguousarray(np.asarray(w, dtype=np.float32))
    b = np.ascontiguousarray(np.asarray(b, dtype=np.float32))

    nc = _get_nc()
    mpc = N_TOKENS // N_CORES
    in_maps = [
        {"x": x[c * mpc:(c + 1) * mpc], "w": w, "b": b} for c in range(N_CORES)
    ]
    res = run_bass_kernel_spmd(nc, in_maps, list(range(N_CORES)))
    LAST_RESULT = res
    return np.concatenate([res.results[c]["out"] for c in range(N_CORES)], axis=0)
